# revision 1
# baseline (speedup 1.0000x reference)
"""Trainium2 Bass kernel: ODE-RNN encoder (z0 encoder), data-parallel over batch.

Strategy
--------
- 8 NeuronCores, batch (n_traj=2048) sharded 256/core; weights replicated.
- Activations kept feature-major on chip: tiles are [feature_partition(128), batch(256)].
  Every GEMM is then `out[Nf_chunk, batch] += W[kf_chunk, Nf_chunk].T @ act[kf_chunk, batch]`
  with the natural [K, N] weight layout as the stationary operand and batch=256 as
  the moving free dim (float32r full-rate needs moving >= 256). No transposes at all.
- Time loop: hardware For_i over the 200 (reversed) steps, carries y/s in SBUF.
- Observation mask m = (sum of mask half of x) > 0 is computed by one matmul with a
  constant 0/1 matrix (reduce + broadcast across partitions in one shot).
- Gating is refactored: with g' = (u-1)*m,
    ny   = y_ode + m*(1-u)*(new_state - y_ode) = y_ode - g'*(new_state - y_ode)
    nstd =      s + m*(1-u)*(new_std  -  s)    =      s - g'*(new_std  -  s)
  (the reference's trailing abs calls are no-ops: convex combos of nonnegatives).
"""

import os
import sys

import numpy as np

if "/opt/trn_rl_repo" not in sys.path:
    sys.path.insert(0, "/opt/trn_rl_repo")

import concourse.bacc as bacc
import concourse.bass as bass
import concourse.mybir as mybir
from concourse import tile
from concourse.alu_op_type import AluOpType
from concourse.bass_utils import run_bass_kernel_spmd

F32 = mybir.dt.float32
F32R = mybir.dt.float32r
F16 = mybir.dt.float16
AF = mybir.ActivationFunctionType

B, NT, IN = 2048, 200, 128
LAT, NU, OU = 256, 512, 256
CAT = 2 * LAT + IN  # 640
NCORES = 8
BC = B // NCORES  # 256 trajectories per core

# Matmul operand dtype. fp16: single-pass PE + FastWeightLoad (fp32r runs a
# 2-pass HI/LO matmul and its 4-byte LDWEIGHTS, ~242ns each, becomes the
# bottleneck: measured 4.9ms of LDWEIGHTS in a 5.5ms kernel).
MM_DT = F16

_last_results = None  # BassKernelResults of the most recent run (for test harness)


def _mm(ap):
    """Matmul operand view (tiles already carry MM_DT)."""
    return ap


class _Bacc(bacc.Bacc):
    def insert_act_table_loads(self):
        import concourse.mybir as mb
        from concourse.bacc import _bass_rust
        from concourse.hw_specs import get_activation_tables

        has_activation = any(
            isinstance(i, mb.InstActivation)
            for b in self.main_func.blocks
            for i in b.instructions
        )
        if not has_activation:
            return
        tables = []
        for name, funcs in get_activation_tables(self.m.arch).items():
            # keep positions (act_func_set_id is positional) but only let
            # sigmoid_and_others match, so one load covers the whole loop
            tables.append((name, funcs if name == "sigmoid_and_others" else set()))
        _bass_rust.insert_act_table_loads(self, tables)


def build_program(nt: int = NT):
    """Build the single-core SPMD Bass program. Returns (nc, input_names)."""
    nc = _Bacc(
        trn_type="TRN2",
        target_bir_lowering=False,
        debug=False,
        enable_asserts=False,
    )

    d = {}
    def inp(name, shape, dt=F32):
        d[name] = nc.dram_tensor(name, shape, dt, kind="ExternalInput").ap()
        return d[name]

    # Per-core data: reversed-time, feature-major x: row t*128+p, col b.
    xs_d = inp("xs", [nt * IN, BC], MM_DT)
    dtsb_d = inp("dtsb", [128, nt])          # dt broadcast along partitions
    dtbo2_d = inp("dtbo2", [128, 2 * nt])    # bo2[c*128+p] * dt[t] at col c*nt+t
    maskw_d = inp("maskw", [128, 128], MM_DT)       # rows 64..127 ones -> mask-half sum bcast

    wo1_d = inp("wo1", [LAT, OU], MM_DT); wo2_d = inp("wo2", [OU, LAT], MM_DT)
    wu1_d = inp("wu1", [CAT, NU], MM_DT); wu2_d = inp("wu2", [NU, LAT], MM_DT)
    wr1_d = inp("wr1", [CAT, NU], MM_DT); wr2_d = inp("wr2", [NU, LAT], MM_DT)
    wn1_d = inp("wn1", [CAT, NU], MM_DT); wn2_d = inp("wn2", [NU, 2 * LAT], MM_DT)
    wt1_d = inp("wt1", [2 * LAT, 100], MM_DT); wt2_d = inp("wt2", [100, 2 * LAT], MM_DT)

    # Bias tiles in per-partition-column layout: [128, n_chunks], col j = b[j*128:(j+1)*128]
    bo1_d = inp("bo1c", [128, 2])
    bu1_d = inp("bu1c", [128, 4]); bu2_d = inp("bu2c", [128, 2])
    br1_d = inp("br1c", [128, 4]); br2_d = inp("br2c", [128, 2])
    bn1_d = inp("bn1c", [128, 4]); bn2_d = inp("bn2c", [128, 4])
    bt1_d = inp("bt1c", [100, 1]); bt2_d = inp("bt2c", [128, 4])

    om_d = nc.dram_tensor("out_mean", [LAT, BC], F32, kind="ExternalOutput").ap()
    os_d = nc.dram_tensor("out_std", [LAT, BC], F32, kind="ExternalOutput").ap()

    with tile.TileContext(nc) as tc:
        with (
            tc.tile_pool(name="wpool", bufs=1) as wpool,
            tc.tile_pool(name="cpool", bufs=1) as cpool,
            tc.tile_pool(name="spool", bufs=3) as spool,
            tc.tile_pool(name="pspool", bufs=8, space=bass.MemorySpace.PSUM) as pspool,
        ):
            def load_w(name, dram, k, n):
                """Load [k,n] weight as k//128 SBUF tiles of [128, n]."""
                tiles = []
                nk = (k + 127) // 128
                for kf in range(nk):
                    p = min(128, k - kf * 128)
                    t = wpool.tile([p, n], MM_DT, name=f"{name}{kf}", tag=f"{name}{kf}")
                    nc.sync.dma_start(t[:], dram[kf * 128 : kf * 128 + p, :])
                    tiles.append(t)
                return tiles

            def load_c(name, dram, p, n, dt=F32):
                t = wpool.tile([p, n], dt, name=name, tag=name)
                nc.sync.dma_start(t[:], dram[:])
                return t

            wo1 = load_w("wo1", wo1_d, LAT, OU)
            wo2 = load_w("wo2", wo2_d, OU, LAT)
            wu1 = load_w("wu1", wu1_d, CAT, NU)
            wu2 = load_w("wu2", wu2_d, NU, LAT)
            wr1 = load_w("wr1", wr1_d, CAT, NU)
            wr2 = load_w("wr2", wr2_d, NU, LAT)
            wn1 = load_w("wn1", wn1_d, CAT, NU)
            wn2 = load_w("wn2", wn2_d, NU, 2 * LAT)
            wt1 = load_w("wt1", wt1_d, 2 * LAT, 100)
            wt2 = load_w("wt2", wt2_d, 100, 2 * LAT)

            bo1 = load_c("bo1", bo1_d, 128, 2)
            bu1 = load_c("bu1", bu1_d, 128, 4)
            bu2 = load_c("bu2", bu2_d, 128, 2)
            br1 = load_c("br1", br1_d, 128, 4)
            br2 = load_c("br2", br2_d, 128, 2)
            bn1 = load_c("bn1", bn1_d, 128, 4)
            bn2 = load_c("bn2", bn2_d, 128, 4)
            bt1 = load_c("bt1", bt1_d, 100, 1)
            bt2 = load_c("bt2", bt2_d, 128, 4)
            dtsb = load_c("dtsb", dtsb_d, 128, nt)
            dtbo2 = load_c("dtbo2", dtbo2_d, 128, 2 * nt)
            maskw = load_c("maskw", maskw_d, 128, 128, MM_DT)

            # Carries (feature-major): y, s as 2 chunks of [128, BC] each.
            ys = [cpool.tile([128, BC], MM_DT, name=f"carry_y{c}", tag=f"y{c}") for c in range(2)]
            ss = [cpool.tile([128, BC], MM_DT, name=f"carry_s{c}", tag=f"s{c}") for c in range(2)]
            for t in (*ys, *ss):
                nc.vector.memset(t[:], 0.0)

            def matgroup(w_tiles, rhs_tiles, n_out_chunks, tag):
                """psum[nf] = sum_kf W[kf][:, nf_chunk].T @ rhs[kf]; returns psum tiles."""
                ps = []
                nk = len(w_tiles)
                for nf in range(n_out_chunks):
                    p = pspool.tile([128, BC], F32, name="ps", tag="ps")
                    for kf in range(nk):
                        nc.tensor.matmul(
                            p[:, :],
                            _mm(w_tiles[kf][:, nf * 128 : nf * 128 + 128]),
                            _mm(rhs_tiles[kf][:]),
                            start=(kf == 0),
                            stop=(kf == nk - 1),
                        )
                    ps.append(p)
                return ps

            def step(iv):
                TT = nc.vector.tensor_tensor
                TS = nc.vector.tensor_scalar
                STT = nc.vector.scalar_tensor_tensor

                xt = spool.tile([128, BC], MM_DT, name="xt", tag="xt")
                nc.sync.dma_start(xt[:], xs_d[bass.ts(iv, 128), :])

                # Observation mask, broadcast along partitions: one matmul + compare.
                mps = pspool.tile([128, BC], F32, name="ps", tag="ps")
                nc.tensor.matmul(mps[:], _mm(maskw[:]), _mm(xt[:]), start=True, stop=True)
                mb = spool.tile([128, BC], F16, name="mb", tag="mb")
                TS(mb[:], mps[:], 0.0, None, AluOpType.is_gt, AluOpType.bypass)

                # ODE mlp: y_ode = y + dt * (tanh(y@Wo1+bo1)@Wo2 + bo2)
                ps1 = matgroup(wo1, ys, 2, "o1")
                ho = []
                for nf in range(2):
                    h = spool.tile([128, BC], MM_DT, name=f"ho{nf}", tag=f"ho{nf}")
                    nc.scalar.activation(h[:], ps1[nf][:], AF.Tanh, bias=bo1[:, nf : nf + 1])
                    ho.append(h)
                ps2 = matgroup(wo2, ho, 2, "o2")
                yo = []
                for nf in range(2):
                    od = spool.tile([128, BC], F16, name=f"od{nf}", tag=f"od{nf}")
                    # od = psum*dt + bo2*dt
                    TS(od[:], ps2[nf][:], dtsb[:, bass.ds(iv, 1)],
                       dtbo2[:, bass.ds(iv + nf * nt, 1)], AluOpType.mult, AluOpType.add)
                    t = spool.tile([128, BC], MM_DT, name=f"yo{nf}", tag=f"yo{nf}")
                    TT(t[:], ys[nf][:], od[:], AluOpType.add)
                    yo.append(t)

                # s/x chunks first: these matmuls can start while the ODE
                # path is still computing yo, keeping the PE warm.
                yc = [ss[0], ss[1], xt, yo[0], yo[1]]
                wu1o = [wu1[2], wu1[3], wu1[4], wu1[0], wu1[1]]
                wr1o = [wr1[2], wr1[3], wr1[4], wr1[0], wr1[1]]

                # u gate
                psu = matgroup(wu1o, yc, 4, "u1")
                hu = []
                for nf in range(4):
                    h = spool.tile([128, BC], MM_DT, name=f"hu{nf}", tag=f"hu{nf}")
                    nc.scalar.activation(h[:], psu[nf][:], AF.Tanh, bias=bu1[:, nf : nf + 1])
                    hu.append(h)
                # r gate first layer (independent of u path; keeps PE busy)
                psr = matgroup(wr1o, yc, 4, "r1")
                hr = []
                for nf in range(4):
                    h = spool.tile([128, BC], MM_DT, name=f"hr{nf}", tag=f"hr{nf}")
                    nc.scalar.activation(h[:], psr[nf][:], AF.Tanh, bias=br1[:, nf : nf + 1])
                    hr.append(h)

                psu2 = matgroup(wu2, hu, 2, "u2")
                gs = []
                for nf in range(2):
                    u = spool.tile([128, BC], F16, name=f"u{nf}", tag=f"u{nf}")
                    nc.scalar.activation(u[:], psu2[nf][:], AF.Sigmoid, bias=bu2[:, nf : nf + 1])
                    # g' = (u - 1) * m   (= -(1-u)*m)
                    g = spool.tile([128, BC], F16, name=f"g{nf}", tag=f"g{nf}")
                    STT(g[:], u[:], 1.0, mb[:], AluOpType.subtract, AluOpType.mult)
                    gs.append(g)

                psr2 = matgroup(wr2, hr, 2, "r2")
                yr, sr = [], []
                for nf in range(2):
                    r = spool.tile([128, BC], F16, name=f"r{nf}", tag=f"r{nf}")
                    nc.scalar.activation(r[:], psr2[nf][:], AF.Sigmoid, bias=br2[:, nf : nf + 1])
                    a = spool.tile([128, BC], MM_DT, name=f"yr{nf}", tag=f"yr{nf}")
                    TT(a[:], yo[nf][:], r[:], AluOpType.mult)
                    yr.append(a)
                    b2 = spool.tile([128, BC], MM_DT, name=f"sr{nf}", tag=f"sr{nf}")
                    TT(b2[:], ss[nf][:], r[:], AluOpType.mult)
                    sr.append(b2)

                c2 = [xt, yr[0], yr[1], sr[0], sr[1]]
                wn1o = [wn1[4], wn1[0], wn1[1], wn1[2], wn1[3]]
                psn = matgroup(wn1o, c2, 4, "n1")
                hn = []
                for nf in range(4):
                    h = spool.tile([128, BC], MM_DT, name=f"hn{nf}", tag=f"hn{nf}")
                    nc.scalar.activation(h[:], psn[nf][:], AF.Tanh, bias=bn1[:, nf : nf + 1])
                    hn.append(h)
                psn2 = matgroup(wn2, hn, 4, "n2")

                # new_state chunks 0..1: ny = y_ode - g'*((psum+bn2) - y_ode)  -> into y carry
                for nf in range(2):
                    dd = spool.tile([128, BC], F16, name=f"d{nf}", tag=f"d{nf}")
                    STT(dd[:], psn2[nf][:], bn2[:, nf : nf + 1], yo[nf][:],
                        AluOpType.add, AluOpType.subtract)
                    t2 = spool.tile([128, BC], F16, name=f"t{nf}", tag=f"t{nf}")
                    TT(t2[:], gs[nf][:], dd[:], AluOpType.mult)
                    TT(ys[nf][:], yo[nf][:], t2[:], AluOpType.subtract)
                # new_std chunks 2..3: nstd = s - g'*(|psum+bn2| - s)  -> into s carry
                for nf in range(2):
                    ab = spool.tile([128, BC], F16, name=f"ab{nf}", tag=f"ab{nf}")
                    nc.scalar.activation(ab[:], psn2[2 + nf][:], AF.Abs,
                                         bias=bn2[:, 2 + nf : 3 + nf])
                    d2 = spool.tile([128, BC], F16, name=f"d2{nf}", tag=f"d2{nf}")
                    TT(d2[:], ab[:], ss[nf][:], AluOpType.subtract)
                    t3 = spool.tile([128, BC], F16, name=f"t3{nf}", tag=f"t3{nf}")
                    TT(t3[:], gs[nf][:], d2[:], AluOpType.mult)
                    TT(ss[nf][:], ss[nf][:], t3[:], AluOpType.subtract)

            tc.For_i_unrolled_general(
                0, nt, 1,
                lambda iv0, unroll: [step(iv0 + i) for i in range(unroll)],
                max_unroll=8,
                hint_engines=(mybir.EngineType.PE,),
            )

            # Final head: z = tanh([y,s]@Wt1+bt1)@Wt2 + bt2
            z1 = pspool.tile([128, BC], F32, name="ps", tag="ps")
            cats = [ys[0], ys[1], ss[0], ss[1]]
            for kf in range(4):
                nc.tensor.matmul(
                    z1[:100, :], _mm(wt1[kf][:, 0:100]), _mm(cats[kf][:]),
                    start=(kf == 0), stop=(kf == 3),
                )
            h1 = spool.tile([100, BC], MM_DT, name="h1", tag="h1")
            nc.scalar.activation(h1[:], z1[:100, :], AF.Tanh, bias=bt1[:, 0:1])
            for nf in range(4):
                zp = pspool.tile([128, BC], F32, name="ps", tag="ps")
                nc.tensor.matmul(
                    zp[:], _mm(wt2[0][:, nf * 128 : nf * 128 + 128]), _mm(h1[:]),
                    start=True, stop=True,
                )
                o = spool.tile([128, BC], F32, name=f"zo{nf}", tag=f"zo{nf}")
                if nf < 2:
                    nc.vector.tensor_scalar(o[:], zp[:], bt2[:, nf : nf + 1], None,
                                            AluOpType.add, AluOpType.bypass)
                    nc.sync.dma_start(om_d[nf * 128 : nf * 128 + 128, :], o[:])
                else:
                    nc.scalar.activation(o[:], zp[:], AF.Abs, bias=bt2[:, nf : nf + 1])
                    oc = spool.tile([128, BC], F32, name=f"zc{nf}", tag=f"zc{nf}")
                    nc.vector.tensor_scalar_max(oc[:], o[:], 1e-20)
                    nc.sync.dma_start(os_d[(nf - 2) * 128 : (nf - 2) * 128 + 128, :], oc[:])

    nc.compile()  # bacc: wait legalization (<=1 wait/inst), reg alloc, DCE
    return nc, list(d.keys())


def make_inputs(data, time_steps, Wu1, bu1, Wu2, bu2, Wr1, br1, Wr2, br2,
                Wn1, bn1, Wn2, bn2, Wo1, bo1, Wo2, bo2, Wt1, bt1, Wt2, bt2,
                nt=None, ncores=NCORES):
    """Host-side shard/layout prep. Returns list of per-core input dicts."""
    f = np.float32
    data = np.asarray(data, f)
    time_steps = np.asarray(time_steps, f)
    nt = data.shape[1] if nt is None else nt

    # Reversed-time Euler dts (see reference): first -0.01, then t[i]-t[i+1] reversed.
    dts = np.concatenate([np.array([-0.01], f),
                          (time_steps[:-1] - time_steps[1:])[::-1]]).astype(f)
    assert dts.shape[0] == nt

    dtsb = np.broadcast_to(dts[None, :], (128, nt)).astype(f).copy()
    bo2c2 = np.asarray(bo2, f).reshape(2, 128)  # chunk-major
    dtbo2 = np.empty((128, 2 * nt), f)
    for c in range(2):
        dtbo2[:, c * nt : (c + 1) * nt] = bo2c2[c][:, None] * dts[None, :]

    maskw = np.zeros((128, 128), f)
    maskw[64:, :] = 1.0

    def bcols(b, p=128):
        b = np.asarray(b, f)
        n = b.shape[0]
        if n % p != 0:
            return b.reshape(n, 1)
        return b.reshape(n // p, p).T.copy()

    h = np.float16 if MM_DT is F16 else f
    shared = dict(
        dtsb=dtsb, dtbo2=dtbo2, maskw=maskw.astype(h),
        wo1=np.asarray(Wo1, h), wo2=np.asarray(Wo2, h),
        wu1=np.asarray(Wu1, h), wu2=np.asarray(Wu2, h),
        wr1=np.asarray(Wr1, h), wr2=np.asarray(Wr2, h),
        wn1=np.asarray(Wn1, h), wn2=np.asarray(Wn2, h),
        wt1=np.asarray(Wt1, h), wt2=np.asarray(Wt2, h),
        bo1c=bcols(bo1), bu1c=bcols(bu1), bu2c=bcols(bu2),
        br1c=bcols(br1), br2c=bcols(br2), bn1c=bcols(bn1), bn2c=bcols(bn2),
        bt1c=bcols(bt1), bt2c=bcols(bt2),
    )

    bc = data.shape[0] // ncores
    # xs[t*128+p, b] = data[b0+b, nt-1-t, p]
    xs_full = np.ascontiguousarray(data[:, ::-1, :].transpose(1, 2, 0))  # [nt, IN, B]
    in_maps = []
    for c in range(ncores):
        xs = np.ascontiguousarray(
            xs_full[:, :, c * bc : (c + 1) * bc]).reshape(nt * IN, bc).astype(h)
        in_maps.append({**shared, "xs": xs})
    return in_maps


def kernel(**inputs):
    """Full-input entry point: shards over 8 cores, runs the Bass kernel, gathers."""
    global _last_results
    nc, _ = build_program(NT)
    in_maps = make_inputs(**inputs)
    res = run_bass_kernel_spmd(nc, in_maps, core_ids=list(range(NCORES)))
    _last_results = res
    mean = np.concatenate([r["out_mean"] for r in res.results], axis=1)  # [LAT, B]
    std = np.concatenate([r["out_std"] for r in res.results], axis=1)
    return mean.T[None].astype(np.float32), std.T[None].astype(np.float32)

